# revision 13
# baseline (speedup 1.0000x reference)
"""Trainium2 Bass kernel for KroneckerLinear: out = x @ kron(f1,f2).T + bias.

Full-input contract: kernel(**inputs) takes the complete x [2097152, 64],
factor1 [8,8], factor2 [8,8], bias [64], returns the full [2097152, 64]
output. Internally shards x row-wise across 8 NeuronCores (data parallel),
replicating the tiny weight/bias to every core.

Per-core algorithm (memory-bound target):
  - View the row shard [262144, 64] as x2 [131072, 128] (pairs of rows),
    so the 64x64 weight becomes a 128x128 block-diagonal W2 = diag(wT, wT)
    and every matmul runs with a full 128-deep contraction.
  - x2 is loaded in [128, 2048] SBUF tiles whose partition lines are 8 KiB
    contiguous DRAM reads (partition p holds 16 consecutive x2 rows).
  - TensorE cannot contract along the free dim, so each [128,128] chunk is
    first transposed on TensorE (matmul-with-identity), copied PSUM->SBUF
    by ScalarE, then fed back as the stationary operand of the real matmul
    with W2 streamed as the moving operand -> natural-layout output chunk.
  - VectorE adds the (free-dim broadcast) bias while copying PSUM->SBUF,
    and the output DMAs back with the same 8 KiB-contiguous line layout.
"""

import numpy as np
from contextlib import ExitStack

from concourse import bacc, bass, mybir, tile
from concourse.bass_utils import run_bass_kernel_spmd

N_CORES = 8
N_ROWS = 2097152
D = 64

R = N_ROWS // N_CORES  # rows per core = 262144
R2 = R // 2  # packed rows per core = 131072 (x2 is [R2, 128])
F2 = 128  # packed feature dim
TILE_ROWS = 2048  # x2 rows per DMA tile (1 MiB)
L = TILE_ROWS // 128  # consecutive x2 rows per partition = 16
N_TILES = R2 // TILE_ROWS  # 64
CHUNK = 128  # x2 rows per matmul chunk
CHUNKS_PER_TILE = TILE_ROWS // CHUNK  # 16
SUBGROUP = 4  # chunks per transpose-PSUM tile (1 bank)
OUTGROUP = 8  # chunks per output-PSUM tile (2 banks)

FP = mybir.dt.float32

_CACHE = {}


def _build_nc(r2=R2, repeat=1, fori=None):
    # Bacc (not plain Bass): its compile() legalizes semaphore waits --
    # TRN2 instructions hold at most one wait, the rest must be split onto
    # standalone EventSemaphore instructions.
    nc = bacc.Bacc("TRN2", target_bir_lowering=False, debug=False)

    n_tiles = r2 // TILE_ROWS
    x2 = nc.dram_tensor("x2", [r2, F2], FP, kind="ExternalInput")
    w2 = nc.dram_tensor("w2", [128, 128], FP, kind="ExternalInput")
    bias_big = nc.dram_tensor("bias_big", [128, OUTGROUP * 128], FP,
                              kind="ExternalInput")
    ident = nc.dram_tensor("ident", [128, 128], FP, kind="ExternalInput")
    out2 = nc.dram_tensor("out2", [r2, F2], FP, kind="ExternalOutput")

    with ExitStack() as ctx:
        tc = ctx.enter_context(tile.TileContext(nc))

        consts = ctx.enter_context(tc.tile_pool(name="consts", bufs=1))
        w2_sb = consts.tile([128, 128], FP)
        nc.sync.dma_start(w2_sb[:], w2[:, :])
        id_sb = consts.tile([128, 128], FP)
        nc.sync.dma_start(id_sb[:], ident[:, :])
        bias_sb = consts.tile([128, OUTGROUP * 128], FP)
        nc.sync.dma_start(bias_sb[:], bias_big[:, :])

        in_pool = ctx.enter_context(tc.tile_pool(name="in_pool", bufs=4))
        xt_psum = ctx.enter_context(
            tc.tile_pool(name="xt_psum", bufs=3, space="PSUM"))
        xt_sb = ctx.enter_context(tc.tile_pool(name="xt_sb", bufs=4))
        out_psum = ctx.enter_context(
            tc.tile_pool(name="out_psum", bufs=2, space="PSUM"))
        out_pool = ctx.enter_context(tc.tile_pool(name="out_pool", bufs=4))

        for t in range(n_tiles * repeat):
            t = t % n_tiles
            r0 = t * TILE_ROWS
            # partition p <- x2 rows [r0 + p*L, r0 + (p+1)*L): 8 KiB lines
            in_ap = x2[r0:r0 + TILE_ROWS, :].rearrange(
                "(p l) f -> p (l f)", p=128)
            in_t = in_pool.tile([128, TILE_ROWS], FP)
            nc.sync.dma_start(in_t[:], in_ap)

            out_t = out_pool.tile([128, TILE_ROWS], FP)

            for og in range(CHUNKS_PER_TILE // OUTGROUP):
                op = out_psum.tile([128, OUTGROUP * 128], FP)
                for sg in range(OUTGROUP // SUBGROUP):
                    xtp = xt_psum.tile([128, SUBGROUP * 128], FP)
                    for j in range(SUBGROUP):
                        c = og * OUTGROUP + sg * SUBGROUP + j
                        nc.tensor.transpose(
                            xtp[:, j * 128:(j + 1) * 128],
                            in_t[:, c * 128:(c + 1) * 128],
                            id_sb[:],
                        )
                    xts = xt_sb.tile([128, SUBGROUP * 128], FP)
                    nc.scalar.copy(xts[:], xtp[:])
                    for j in range(SUBGROUP):
                        k = sg * SUBGROUP + j
                        nc.tensor.matmul(
                            op[:, k * 128:(k + 1) * 128],
                            xts[:, j * 128:(j + 1) * 128],
                            w2_sb[:],
                            start=True,
                            stop=True,
                        )
                nc.vector.tensor_add(
                    out_t[:, og * OUTGROUP * 128:(og + 1) * OUTGROUP * 128],
                    op[:],
                    bias_sb[:],
                )

            out_ap = out2[r0:r0 + TILE_ROWS, :].rearrange(
                "(p l) f -> p (l f)", p=128)
            nc.scalar.dma_start(out_ap, out_t[:])

    nc.compile()
    return nc


def _get_nc():
    if "nc" not in _CACHE:
        _CACHE["nc"] = _build_nc()
    return _CACHE["nc"]


def _prep_in_maps(x, factor1, factor2, bias):
    x = np.ascontiguousarray(np.asarray(x, dtype=np.float32))
    w = np.kron(np.asarray(factor1, np.float32),
                np.asarray(factor2, np.float32))  # [64, 64]
    w2 = np.zeros((128, 128), dtype=np.float32)
    w2[:64, :64] = w.T
    w2[64:, 64:] = w.T
    b2 = np.concatenate([np.asarray(bias, np.float32)] * 2)  # [128]
    bias_big = np.ascontiguousarray(
        np.broadcast_to(np.tile(b2, OUTGROUP), (128, OUTGROUP * 128)))
    ident = np.eye(128, dtype=np.float32)

    x2 = x.reshape(N_ROWS // 2, F2)
    in_maps = []
    for c in range(N_CORES):
        shard = np.ascontiguousarray(x2[c * R2:(c + 1) * R2])
        in_maps.append({
            "x2": shard,
            "w2": w2,
            "bias_big": bias_big,
            "ident": ident,
        })
    return in_maps


def run(inputs, trace=False, **run_kwargs):
    """Returns (full_output, BassKernelResults)."""
    nc = _get_nc()
    in_maps = _prep_in_maps(**inputs)
    res = run_bass_kernel_spmd(nc, in_maps, list(range(N_CORES)),
                               trace=trace, **run_kwargs)
    shards = [np.asarray(res.results[c]["out2"]) for c in range(N_CORES)]
    out = np.concatenate(shards, axis=0).reshape(N_ROWS, D)
    return out, res


def kernel(x, factor1, factor2, bias):
    out, _ = run(dict(x=x, factor1=factor1, factor2=factor2, bias=bias))
    return out.astype(np.float32)


# revision 15
# speedup vs baseline: 9.9406x; 9.9406x over previous
"""Trainium2 Bass kernel for KroneckerLinear: out = x @ kron(f1,f2).T + bias.

Full-input contract: kernel(**inputs) takes the complete x [2097152, 64],
factor1 [8,8], factor2 [8,8], bias [64], returns the full [2097152, 64]
output. Internally shards x row-wise across 8 NeuronCores (data parallel),
replicating the tiny weight/bias to every core.

Per-core algorithm (memory-bound target):
  - View the row shard [262144, 64] as x2 [131072, 128] (pairs of rows),
    so the 64x64 weight becomes a 128x128 block-diagonal W2 = diag(wT, wT)
    and every matmul runs with a full 128-deep contraction.
  - x2 is loaded in [128, 2048] SBUF tiles whose partition lines are 8 KiB
    contiguous DRAM reads (partition p holds 16 consecutive x2 rows).
  - TensorE cannot contract along the free dim, so each [128,128] chunk is
    first transposed on TensorE (matmul-with-identity), copied PSUM->SBUF
    by ScalarE, then fed back as the stationary operand of the real matmul
    with W2 streamed as the moving operand -> natural-layout output chunk.
  - VectorE adds the (free-dim broadcast) bias while copying PSUM->SBUF,
    and the output DMAs back with the same 8 KiB-contiguous line layout.
"""

import numpy as np
from contextlib import ExitStack

from concourse import bacc, bass, mybir, tile
from concourse.bass_utils import run_bass_kernel_spmd

N_CORES = 8
N_ROWS = 2097152
D = 64

R = N_ROWS // N_CORES  # rows per core = 262144
R2 = R // 2  # packed rows per core = 131072 (x2 is [R2, 128])
F2 = 128  # packed feature dim
TILE_ROWS = 2048  # x2 rows per DMA tile (1 MiB)
L = TILE_ROWS // 128  # consecutive x2 rows per partition = 16
N_TILES = R2 // TILE_ROWS  # 64
CHUNK = 128  # x2 rows per matmul chunk
CHUNKS_PER_TILE = TILE_ROWS // CHUNK  # 16
SUBGROUP = 4  # chunks per transpose-PSUM tile (1 bank)
OUTGROUP = 8  # chunks per output-PSUM tile (2 banks)

FP = mybir.dt.float32

_CACHE = {}


def _build_nc(r2=R2, repeat=1, fori=None):
    # Bacc (not plain Bass): its compile() legalizes semaphore waits --
    # TRN2 instructions hold at most one wait, the rest must be split onto
    # standalone EventSemaphore instructions.
    nc = bacc.Bacc("TRN2", target_bir_lowering=False, debug=False)

    n_tiles = r2 // TILE_ROWS
    x2 = nc.dram_tensor("x2", [r2, F2], FP, kind="ExternalInput")
    w2 = nc.dram_tensor("w2", [128, 128], FP, kind="ExternalInput")
    bias_big = nc.dram_tensor("bias_big", [128, OUTGROUP * 128], FP,
                              kind="ExternalInput")
    ident = nc.dram_tensor("ident", [128, 128], FP, kind="ExternalInput")
    out2 = nc.dram_tensor("out2", [r2, F2], FP, kind="ExternalOutput")

    with ExitStack() as ctx:
        tc = ctx.enter_context(tile.TileContext(nc))

        consts = ctx.enter_context(tc.tile_pool(name="consts", bufs=1))
        w2_sb = consts.tile([128, 128], FP)
        nc.sync.dma_start(w2_sb[:], w2[:, :])
        id_sb = consts.tile([128, 128], FP)
        nc.sync.dma_start(id_sb[:], ident[:, :])
        bias_sb = consts.tile([128, OUTGROUP * 128], FP)
        nc.sync.dma_start(bias_sb[:], bias_big[:, :])

        in_pool = ctx.enter_context(tc.tile_pool(name="in_pool", bufs=4))
        xt_psum = ctx.enter_context(
            tc.tile_pool(name="xt_psum", bufs=3, space="PSUM"))
        xt_sb = ctx.enter_context(tc.tile_pool(name="xt_sb", bufs=4))
        out_psum = ctx.enter_context(
            tc.tile_pool(name="out_psum", bufs=2, space="PSUM"))
        out_pool = ctx.enter_context(tc.tile_pool(name="out_pool", bufs=4))

        loop_ctx = tc.For_i(0, fori, 1) if fori is not None else None
        if loop_ctx is not None:
            loop_ctx.__enter__()

        for t in range(n_tiles * repeat):
            t = t % n_tiles
            r0 = t * TILE_ROWS
            # partition p <- x2 rows [r0 + p*L, r0 + (p+1)*L): 8 KiB lines
            in_ap = x2[r0:r0 + TILE_ROWS, :].rearrange(
                "(p l) f -> p (l f)", p=128)
            in_t = in_pool.tile([128, TILE_ROWS], FP)
            nc.sync.dma_start(in_t[:], in_ap)

            out_t = out_pool.tile([128, TILE_ROWS], FP)

            for og in range(CHUNKS_PER_TILE // OUTGROUP):
                op = out_psum.tile([128, OUTGROUP * 128], FP)
                for sg in range(OUTGROUP // SUBGROUP):
                    xtp = xt_psum.tile([128, SUBGROUP * 128], FP)
                    for j in range(SUBGROUP):
                        c = og * OUTGROUP + sg * SUBGROUP + j
                        nc.tensor.transpose(
                            xtp[:, j * 128:(j + 1) * 128],
                            in_t[:, c * 128:(c + 1) * 128],
                            id_sb[:],
                        )
                    xts = xt_sb.tile([128, SUBGROUP * 128], FP)
                    nc.scalar.copy(xts[:], xtp[:])
                    for j in range(SUBGROUP):
                        k = sg * SUBGROUP + j
                        nc.tensor.matmul(
                            op[:, k * 128:(k + 1) * 128],
                            xts[:, j * 128:(j + 1) * 128],
                            w2_sb[:],
                            start=True,
                            stop=True,
                        )
                nc.vector.tensor_add(
                    out_t[:, og * OUTGROUP * 128:(og + 1) * OUTGROUP * 128],
                    op[:],
                    bias_sb[:],
                )

            out_ap = out2[r0:r0 + TILE_ROWS, :].rearrange(
                "(p l) f -> p (l f)", p=128)
            nc.scalar.dma_start(out_ap, out_t[:])

        if loop_ctx is not None:
            loop_ctx.__exit__(None, None, None)

    nc.compile()
    return nc


def _get_nc():
    if "nc" not in _CACHE:
        _CACHE["nc"] = _build_nc()
    return _CACHE["nc"]


def _prep_in_maps(x, factor1, factor2, bias):
    x = np.ascontiguousarray(np.asarray(x, dtype=np.float32))
    w = np.kron(np.asarray(factor1, np.float32),
                np.asarray(factor2, np.float32))  # [64, 64]
    w2 = np.zeros((128, 128), dtype=np.float32)
    w2[:64, :64] = w.T
    w2[64:, 64:] = w.T
    b2 = np.concatenate([np.asarray(bias, np.float32)] * 2)  # [128]
    bias_big = np.ascontiguousarray(
        np.broadcast_to(np.tile(b2, OUTGROUP), (128, OUTGROUP * 128)))
    ident = np.eye(128, dtype=np.float32)

    x2 = x.reshape(N_ROWS // 2, F2)
    in_maps = []
    for c in range(N_CORES):
        shard = np.ascontiguousarray(x2[c * R2:(c + 1) * R2])
        in_maps.append({
            "x2": shard,
            "w2": w2,
            "bias_big": bias_big,
            "ident": ident,
        })
    return in_maps


def run(inputs, trace=False, **run_kwargs):
    """Returns (full_output, BassKernelResults)."""
    nc = _get_nc()
    in_maps = _prep_in_maps(**inputs)
    res = run_bass_kernel_spmd(nc, in_maps, list(range(N_CORES)),
                               trace=trace, **run_kwargs)
    shards = [np.asarray(res.results[c]["out2"]) for c in range(N_CORES)]
    out = np.concatenate(shards, axis=0).reshape(N_ROWS, D)
    return out, res


def kernel(x, factor1, factor2, bias):
    out, _ = run(dict(x=x, factor1=factor1, factor2=factor2, bias=bias))
    return out.astype(np.float32)


# revision 25
# speedup vs baseline: 237.2050x; 23.8623x over previous
"""Trainium2 Bass kernel for KroneckerLinear: out = x @ kron(f1,f2).T + bias.

Full-input contract: kernel(**inputs) takes the complete x [2097152, 64],
factor1 [8,8], factor2 [8,8], bias [64], returns the full [2097152, 64]
output. Internally shards x row-wise across 8 NeuronCores (data parallel),
replicating the tiny weight/bias to every core.

Per-core algorithm (memory-bound target):
  - View the row shard [262144, 64] as x2 [131072, 128] (pairs of rows),
    so the 64x64 weight becomes a 128x128 block-diagonal W2 = diag(wT, wT)
    and every matmul runs with a full 128-deep contraction.
  - x2 is loaded in [128, tile_rows] SBUF tiles whose partition lines are
    contiguous DRAM reads (partition p holds tile_rows/128 consecutive
    x2 rows).
  - TensorE cannot contract along the free dim, so each [128,128] chunk is
    first transposed on TensorE (matmul-with-identity), copied PSUM->SBUF
    by ScalarE, then fed back as the stationary operand of the real matmul
    with W2 streamed as the moving operand -> natural-layout output chunk.
  - VectorE adds the (free-dim broadcast) bias while copying PSUM->SBUF,
    and the output DMAs back with the same contiguous line layout.
The pipeline is DMA-bound (compute fully hidden behind the HBM streams).
"""

import numpy as np
from contextlib import ExitStack

from concourse import bacc, bass, mybir, tile
from concourse.bass_utils import run_bass_kernel_spmd

N_CORES = 8
N_ROWS = 2097152
D = 64

R = N_ROWS // N_CORES  # rows per core = 262144
R2 = R // 2  # packed rows per core = 131072 (x2 is [R2, 128])
F2 = 128  # packed feature dim
TILE_ROWS = 2048  # default x2 rows per DMA tile (1 MiB)
CHUNK = 128  # x2 rows per matmul chunk
SUBGROUP = 4  # chunks per transpose-PSUM tile (1 bank)
OUTGROUP = 8  # chunks per output-PSUM tile (2 banks)
N_TILES = R2 // TILE_ROWS

FP = mybir.dt.float32

_CACHE = {}


def _build_nc(r2=R2, repeat=1, fori=None, variant="full", f32r=False,
              tile_rows=TILE_ROWS, in_bufs=4, out_bufs=4, split_dma=False):
    # Bacc (not plain Bass): its compile() legalizes semaphore waits --
    # TRN2 instructions hold at most one wait; the rest are split onto
    # standalone EventSemaphore instructions by Bacc.compile().
    nc = bacc.Bacc("TRN2", target_bir_lowering=False, debug=False)

    n_tiles = r2 // tile_rows
    chunks_per_tile = tile_rows // CHUNK
    # fp32r double-wide: stream [W2|W2] (N=256 -> 1 cyc/row vs fp32's 4);
    # result duplicated, only the first 128 columns of each 256 are kept.
    og_size = 4 if f32r else OUTGROUP
    ow = 256 if f32r else 128  # psum columns per chunk

    x2 = nc.dram_tensor("x2", [r2, F2], FP, kind="ExternalInput")
    w2 = nc.dram_tensor("w2", [128, 128], FP, kind="ExternalInput")
    bias_big = nc.dram_tensor("bias_big", [128, OUTGROUP * 128], FP,
                              kind="ExternalInput")
    ident = nc.dram_tensor("ident", [128, 128], FP, kind="ExternalInput")
    out2 = nc.dram_tensor("out2", [r2, F2], FP, kind="ExternalOutput")

    FR = mybir.dt.float32r

    with ExitStack() as ctx:
        tc = ctx.enter_context(tile.TileContext(nc))

        consts = ctx.enter_context(tc.tile_pool(name="consts", bufs=1))
        w2_sb = consts.tile([128, 256 if f32r else 128], FP)
        nc.sync.dma_start(w2_sb[:, 0:128], w2[:, :])
        if f32r:
            nc.sync.dma_start(w2_sb[:, 128:256], w2[:, :])
        id_sb = consts.tile([128, 128], FP)
        nc.sync.dma_start(id_sb[:], ident[:, :])
        bias_sb = consts.tile([128, OUTGROUP * 128], FP)
        nc.sync.dma_start(bias_sb[:], bias_big[:, :])

        in_pool = ctx.enter_context(tc.tile_pool(name="in_pool",
                                                 bufs=in_bufs))
        xt_psum = ctx.enter_context(
            tc.tile_pool(name="xt_psum", bufs=3, space="PSUM"))
        xt_sb = ctx.enter_context(tc.tile_pool(name="xt_sb", bufs=4))
        out_psum = ctx.enter_context(
            tc.tile_pool(name="out_psum", bufs=2, space="PSUM"))
        out_pool = ctx.enter_context(tc.tile_pool(name="out_pool",
                                                  bufs=out_bufs))

        loop_ctx = tc.For_i(0, fori, 1) if fori is not None else None
        if loop_ctx is not None:
            loop_ctx.__enter__()

        noio_in = None
        if variant == "noio":
            noio_in = in_pool.tile([128, tile_rows], FP)
            nc.sync.dma_start(
                noio_in[:],
                x2[0:tile_rows, :].rearrange("(p l) f -> p (l f)", p=128))

        for t in range(n_tiles * repeat):
            t = t % n_tiles
            r0 = t * tile_rows
            # partition p <- x2 rows [r0 + p*L, r0 + (p+1)*L): contiguous
            in_ap = x2[r0:r0 + tile_rows, :].rearrange(
                "(p l) f -> p (l f)", p=128)
            out_ap = out2[r0:r0 + tile_rows, :].rearrange(
                "(p l) f -> p (l f)", p=128)
            if variant == "noio":
                in_t = noio_in
            else:
                in_t = in_pool.tile([128, tile_rows], FP)
                if split_dma:
                    h = tile_rows // 2
                    nc.sync.dma_start(in_t[:, 0:h], in_ap[:, 0:h])
                    nc.scalar.dma_start(in_t[:, h:], in_ap[:, h:])
                else:
                    nc.sync.dma_start(in_t[:], in_ap)

            if variant == "dmaonly":
                nc.scalar.dma_start(out_ap, in_t[:])
                continue

            out_t = out_pool.tile([128, tile_rows], FP)

            for og in range(chunks_per_tile // og_size):
                op = out_psum.tile([128, og_size * ow], FP)
                for sg in range(og_size // SUBGROUP):
                    xtp = xt_psum.tile([128, SUBGROUP * 128], FP)
                    for j in range(SUBGROUP):
                        c = og * og_size + sg * SUBGROUP + j
                        nc.tensor.transpose(
                            xtp[:, j * 128:(j + 1) * 128],
                            in_t[:, c * 128:(c + 1) * 128],
                            id_sb[:],
                        )
                    if variant != "pe_only":
                        xts = xt_sb.tile([128, SUBGROUP * 128], FP)
                        nc.scalar.copy(xts[:], xtp[:])
                    for j in range(SUBGROUP):
                        k = sg * SUBGROUP + j
                        lhs = (id_sb[:] if variant == "pe_only"
                               else xts[:, j * 128:(j + 1) * 128])
                        rhs = w2_sb[:]
                        out_sl = op[:, k * ow:k * ow + ow]
                        if f32r:
                            lhs = lhs.bitcast(FR)
                            rhs = rhs.bitcast(FR)
                        nc.tensor.matmul(out_sl, lhs, rhs,
                                         start=True, stop=True)
                if variant != "pe_only":
                    ot_sl = out_t[:, og * og_size * 128:
                                  (og + 1) * og_size * 128]
                    if f32r:
                        # keep the first 128 of each 256-wide result
                        src = op[:].rearrange(
                            "p (k d) -> p k d", d=256)[:, :, 0:128]
                        dst = ot_sl.rearrange("p (k d) -> p k d", d=128)
                        bia = bias_sb[:, 0:og_size * 128].rearrange(
                            "p (k d) -> p k d", d=128)
                        nc.vector.tensor_add(dst, src, bia)
                    else:
                        nc.vector.tensor_add(ot_sl, op[:],
                                             bias_sb[:, 0:og_size * 128])

            if variant not in ("noio", "pe_only") or t == n_tiles - 1:
                if split_dma and variant == "full":
                    h = tile_rows // 2
                    nc.scalar.dma_start(out_ap[:, 0:h], out_t[:, 0:h])
                    nc.sync.dma_start(out_ap[:, h:], out_t[:, h:])
                else:
                    nc.scalar.dma_start(out_ap, out_t[:])

        if loop_ctx is not None:
            loop_ctx.__exit__(None, None, None)

    nc.compile()
    return nc


def _get_nc():
    if "nc" not in _CACHE:
        _CACHE["nc"] = _build_nc(tile_rows=4096, in_bufs=4, out_bufs=4)
    return _CACHE["nc"]


def _prep_in_maps(x, factor1, factor2, bias):
    x = np.ascontiguousarray(np.asarray(x, dtype=np.float32))
    w = np.kron(np.asarray(factor1, np.float32),
                np.asarray(factor2, np.float32))  # [64, 64]
    w2 = np.zeros((128, 128), dtype=np.float32)
    w2[:64, :64] = w.T
    w2[64:, 64:] = w.T
    b2 = np.concatenate([np.asarray(bias, np.float32)] * 2)  # [128]
    bias_big = np.ascontiguousarray(
        np.broadcast_to(np.tile(b2, OUTGROUP), (128, OUTGROUP * 128)))
    ident = np.eye(128, dtype=np.float32)

    x2 = x.reshape(N_ROWS // 2, F2)
    in_maps = []
    for c in range(N_CORES):
        shard = np.ascontiguousarray(x2[c * R2:(c + 1) * R2])
        in_maps.append({
            "x2": shard,
            "w2": w2,
            "bias_big": bias_big,
            "ident": ident,
        })
    return in_maps


def run(inputs, trace=False, **run_kwargs):
    """Returns (full_output, BassKernelResults)."""
    nc = _get_nc()
    in_maps = _prep_in_maps(**inputs)
    try:
        res = run_bass_kernel_spmd(nc, in_maps, list(range(N_CORES)),
                                   trace=trace, **run_kwargs)
    except Exception:
        # One retry: transient device-state failures (e.g. a wedged core
        # from a previous run) usually clear on the next execution.
        res = run_bass_kernel_spmd(nc, in_maps, list(range(N_CORES)),
                                   trace=trace, **run_kwargs)
    shards = [np.asarray(res.results[c]["out2"]) for c in range(N_CORES)]
    out = np.concatenate(shards, axis=0).reshape(N_ROWS, D)
    return out, res


def kernel(x, factor1, factor2, bias):
    out, _ = run(dict(x=x, factor1=factor1, factor2=factor2, bias=bias))
    return out.astype(np.float32)
